# revision 21
# baseline (speedup 1.0000x reference)
"""SAGAN-style self-attention block on 8 trn2 NeuronCores.

Full inputs: x [8, 512, 64, 64], w_theta [64, 512], w_phi [64, 512],
w_g [256, 512], w_o [512, 256], gamma scalar.

Sharding: data-parallel over batch — one batch item per core. Each core runs
an identical Bass program over its own x[b]; weights are replicated.

Per-core math (C=512, n=H*W=4096, m=n/4=1024):
  theta = w_theta @ x            [64, 4096]
  phi   = pool2(w_phi @ x)       [64, 1024]
  g     = pool2(w_g @ x)         [256, 1024]
  S^T   = phi^T @ theta          [1024, 4096]  (scores, transposed layout)
  E     = exp(S^T)               (bf16; no max-subtraction: |S| < ~51)
  Z     = ones^T @ sum-tree(E)   (pair tree: 4 bf16 adds on gpsimd, 3 on
                                  DVE; broadcast row sums via one PE matmul)
  att   = (g @ E) / Z            [256, 4096]   (bf16 after normalize)
  out   = (gamma*w_o) @ att + x  [512, 4096]

The projection runs as fp8 (e4m3) DoubleRow matmuls at 0.5 cycles/column:
x and w_proj are split host-side into hi+lo e4m3 pairs (w scaled by 16, x by
4 so the residuals stay in e4m3's normal range); the three cross terms
wh@xh + wh@xl + wl@xh recover bf16-level accuracy at 75% of the bf16 matmul
cost. The 64x scale cancels exactly: theta drains through an ACT copy with
scale 2^-12, and the Z ones-matmul uses 64.0 so att = (64 g E)/(64 Z).
Scores/attend/out matmuls stay bf16 (fp8 there fails the rel-err budget:
softmax logit noise flips argmaxes at sigma_S ~ 8). g transposes ride the
DMA xbar (dma_start_transpose) instead of PE+DVE.

The residual add uses a separate bf16 copy of x; in the drain (slices 6-7)
half the residuals ride the PE via an identity matmul so the psum drain
becomes an ACT copy. Stores are batched [128,4,512] 3D DMAs.
"""

import time
from contextlib import ExitStack

import numpy as np

import bass_rust
import concourse.bass as bass
import concourse.mybir as mybir
import concourse.tile as tile
from concourse.bass_utils import run_bass_kernel_spmd
from concourse.masks import make_identity

P = 128
C = 512  # channels
C8 = 64  # theta/phi channels
C2 = 256  # g channels
N = 4096  # H*W
M = 1024  # pooled spatial
NS = 8  # n-slices
SL = 512  # n-slice width
MT = 8  # m-tiles of 128
F32 = mybir.dt.float32
BF16 = mybir.dt.bfloat16
E4 = mybir.dt.float8e4
AX = mybir.AxisListType
ALU = mybir.AluOpType
ACTF = mybir.ActivationFunctionType
DR = mybir.MatmulPerfMode.DoubleRow


def _pool_view(ap, w=SL):
    """[p, w] slice of the conv output -> 5D maxpool view [p, h2, w2, dy, dx].

    Within an n-slice of 512 = 8 image rows: local n = (2*h2+dy)*64 + 2*w2+dx.
    """
    return ap.rearrange(
        "p (h2 dy w2 dx) -> p h2 w2 dy dx", h2=w // 128, dy=2, w2=32, dx=2
    )


def emit(nc, tc, ctx):
    xh_d = nc.dram_tensor("xh", [C, N], E4, kind="ExternalInput")
    xl_d = nc.dram_tensor("xl", [C, N], E4, kind="ExternalInput")
    wph_d = nc.dram_tensor("wph", [C, 384], E4, kind="ExternalInput")
    wpl_d = nc.dram_tensor("wpl", [C, 384], E4, kind="ExternalInput")
    wo_d = nc.dram_tensor("wo", [C2, C], BF16, kind="ExternalInput")
    out_d = nc.dram_tensor("out", [C, N], BF16, kind="ExternalOutput")

    persist = ctx.enter_context(tc.tile_pool(name="persist", bufs=1))

    # weights on the scalar (ACT) DMA queue in chunks so chunk 0 lands early:
    # the first projection matmul only waits on it + the first x chunk
    wpth = persist.tile([P, 4, 384], E4, name="wpth")
    wptl = persist.tile([P, 4, 384], E4, name="wptl")
    wph3 = wph_d.ap().rearrange("(k p) o -> p k o", k=4)
    nc.scalar.dma_start(out=wpth[:, 0:2, :], in_=wph3[:, 0:2, :])
    wot = persist.tile([P, 2, C], BF16, name="wot")

    ones_b = persist.tile([P, P], BF16)
    nc.vector.memset(ones_b, 64.0)
    ident_f = persist.tile([P, P], F32)
    make_identity(nc, ident_f)
    ident_b = persist.tile([P, P], BF16)
    nc.vector.tensor_copy(ident_b, ident_f)
    # fp8 DoubleRow identity pair [0.25 I; 0.25 I]: one DR matmul against the
    # [xh; xl] k-pair reconstructs the residual x into the out psum exactly
    # (0.25 undoes the 4x host-side scale on x)
    identq = persist.tile([P, 2, P], E4, name="identq")
    for i in range(2):
        nc.vector.tensor_scalar(
            out=identq[:, i, :], in0=ident_f, scalar1=0.25, scalar2=None,
            op0=ALU.mult,
        )

    # score psum pool lives across both phases (slice 0/1 scores start in
    # phase 1)
    spool = ctx.enter_context(tc.tile_pool(name="spsum", bufs=3, space="PSUM"))
    etp = ctx.enter_context(tc.tile_pool(name="et", bufs=4))
    fsp = ctx.enter_context(tc.tile_pool(name="fs", bufs=2))
    miscp = ctx.enter_context(tc.tile_pool(name="misc", bufs=2))

    # Warm-up for the otherwise-idle startup window (PE waits ~4us for the
    # first weight+x data): dummy exp preloads the ACT exp table, and a burst
    # of matmuls on constant data ramps the PE clock (HAM).
    actwarm = persist.tile([P, 1], F32)
    nc.scalar.activation(actwarm, ident_f[:, 0:1], ACTF.Exp)
    for wi in range(5):
        wt_ = spool.tile([P, P], F32, name="warm", tag="s0", bufs=1)
        nc.tensor.matmul(wt_, lhsT=ones_b, rhs=ones_b, start=True, stop=True)

    # x loads: hi and lo fp8 streams in one [P, 2(hi/lo), 4(k), N] tile so a
    # DoubleRow ap can pair them. hi rides the sync (SP) queue, lo the scalar
    # queue behind the first weight chunk; HWDGE descriptor-gen is a shared
    # serial ~630ns/DMA resource, so loads are batched 2 slices per DMA once
    # the pipeline is primed.
    xhl = persist.tile([P, 2, 4, N], E4, name="xhl")
    xh3 = xh_d.ap().rearrange("(k p) n -> p k n", k=4)
    xl3 = xl_d.ap().rearrange("(k p) n -> p k n", k=4)
    for cc in range(2):
        nc.sync.dma_start(
            out=xhl[:, 0, 2 * cc : 2 * cc + 2, 0:SL],
            in_=xh3[:, 2 * cc : 2 * cc + 2, 0:SL],
        )
    nc.sync.dma_start(out=xhl[:, 0, :, SL : 2 * SL], in_=xh3[:, :, SL : 2 * SL])
    for q in range(1, 4):
        qs = slice(2 * q * SL, 2 * (q + 1) * SL)
        nc.sync.dma_start(out=xhl[:, 0, :, qs], in_=xh3[:, :, qs])
        nc.sync.dma_start(out=xhl[:, 1, :, qs], in_=xl3[:, :, qs])
    nc.sync.dma_start(
        out=wot, in_=wo_d.ap().rearrange("(ct p) c -> p ct c", ct=2)
    )
    # scalar queue interleaves the remaining weight chunks with the early
    # xl slices so every projection term unblocks as soon as possible;
    # (wl, xh) is the last term per slice, so wptl can land latest
    nc.scalar.dma_start(out=xhl[:, 1, :, 0:SL], in_=xl3[:, :, 0:SL])
    nc.scalar.dma_start(out=xhl[:, 1, :, SL : 2 * SL], in_=xl3[:, :, SL : 2 * SL])
    nc.scalar.dma_start(out=wpth[:, 2:4, :], in_=wph3[:, 2:4, :])
    nc.scalar.dma_start(
        out=wptl, in_=wpl_d.ap().rearrange("(k p) o -> p k o", k=4)
    )

    out3 = out_d.ap().rearrange("(ot p) n -> p ot n", ot=4)

    theta = persist.tile([C8, N], BF16)
    phi = persist.tile([C8, M], BF16)
    g01 = persist.tile([P, 2, M], BF16, name="g01")
    gT = [persist.tile([P, C2], BF16, name=f"gT{mt}") for mt in range(MT)]

    ET = [[None] * MT for _ in range(NS)]
    FS1 = [[None] * 4 for _ in range(NS)]  # pair sums
    ZB = [None] * NS  # broadcast row sums
    ATT = [[None, None] for _ in range(NS)]
    score_done = [[False] * MT for _ in range(NS)]
    fs1_done = [[False] * 4 for _ in range(NS)]

    def emit_score(i, mt):
        # S^T tile [m 128, n 512] = phi_mt^T @ theta_i  (K = 64 channels)
        sp = spool.tile([P, SL], F32, name="sp", tag=f"s{mt % 3}", bufs=1)
        nc.tensor.matmul(
            sp,
            lhsT=phi[:, mt * P : (mt + 1) * P],
            rhs=theta[:, i * SL : (i + 1) * SL],
            start=True,
            stop=True,
            skip_group_check=True,
        )
        et = etp.tile([P, SL], BF16, name="et", tag=f"et{mt}")
        nc.scalar.activation(et, sp, ACTF.Exp)
        ET[i][mt] = et
        score_done[i][mt] = True

    def emit_fs1(i, j, eng=None):
        # pair sums on the Pool engine (gpsimd): it cannot touch PSUM, so
        # this SBUF-only stage is the one piece of DVE work it can take
        t = fsp.tile([P, SL], BF16, name="fs1", tag=f"f{j}")
        (eng or nc.gpsimd).tensor_add(t, ET[i][2 * j], ET[i][2 * j + 1])
        FS1[i][j] = t
        fs1_done[i][j] = True

    def emit_fs_rest(i, eng=None):
        # finish the pair-sum tree and hand the total to the Pool engine for
        # the cross-partition broadcast sum
        for j in range(4):
            if not fs1_done[i][j]:
                emit_fs1(i, j, eng)
        h0 = fsp.tile([P, SL], BF16, name="fs2", tag="h0")
        h1 = fsp.tile([P, SL], BF16, name="fs2", tag="h1")
        nc.vector.tensor_add(h0, FS1[i][0], FS1[i][1])
        nc.vector.tensor_add(h1, FS1[i][2], FS1[i][3])
        hh = fsp.tile([P, SL], BF16, name="fs3", tag="hh")
        nc.vector.tensor_add(hh, h0, h1)
        # broadcast row sums via ones-matmul (cross-partition reduce); the
        # 64.0 ones value cancels the 64x projection scale on g. rides a
        # score psum bank that is idle at this point of the iteration
        zb = spool.tile([P, SL], F32, name="zb", tag="s2", bufs=1)
        nc.tensor.matmul(
            zb, lhsT=ones_b, rhs=hh, start=True, stop=True,
            skip_group_check=True,
        )
        ZB[i] = zb

    def emit_scores_full(i):
        for mt in range(MT):
            if not score_done[i][mt]:
                emit_score(i, mt)

    def emit_ap(qp, i, lo, w, mts=range(MT), ap=None, tags=("a0", "a1"), pool=None):
        esl = slice(lo, lo + w)
        if ap is None:
            ap = [
                (pool or qp).tile([P, w], F32, name="ap", tag=tags[ct], bufs=1)
                for ct in range(2)
            ]
        for mt in mts:
            st, sp_ = (mt == 0), (mt == MT - 1)
            for ct in range(2):
                nc.tensor.matmul(
                    ap[ct],
                    lhsT=gT[mt][:, ct * P : (ct + 1) * P],
                    rhs=ET[i][mt][:, esl],
                    start=st,
                    stop=sp_,
                    skip_group_check=True,
                )
        return ap

    def emit_norm(i, ap, lo, w):
        esl = slice(lo, lo + w)
        rinv = miscp.tile([P, w], F32, name="rinv", tag="rinv")
        nc.vector.reciprocal(rinv, ZB[i][:, esl])
        att = [None, None]
        # ct0 first: the out matmuls consume ct0 first
        for ct in (0, 1):
            t = miscp.tile([P, w], BF16, name="att", tag=f"att{ct}")
            nc.vector.tensor_mul(t, ap[ct], rinv)
            att[ct] = t
        ATT[i] = att

    def emit_out(qp, i, lo, w, ots, obt=None, queue=None, store_split=False):
        # project + residual for columns [i*SL+lo, ..+w); the batched store
        # goes out once ot 3 is in (or per ot-pair when store_split)
        nsl = slice(i * SL + lo, i * SL + lo + w)
        att = ATT[i]
        if obt is None:
            # 4-deep: a slice's batched store completes ~1.5 iterations after
            # its compute, and the drain chunks must not wait on it
            tag = "ob" if w == SL else "obc"
            obt = miscp.tile([P, 4, w], BF16, name="ob", tag=tag, bufs=3)
        for ot in ots:
            op_ = qp.tile([P, w], F32, name="op", tag="o", bufs=3)
            for ct in range(2):
                nc.tensor.matmul(
                    op_,
                    lhsT=wot[:, ct, ot * P : (ot + 1) * P],
                    rhs=att[ct],
                    start=(ct == 0),
                    stop=False,
                    skip_group_check=True,
                )
            # residual: one fp8 DoubleRow identity matmul accumulates
            # 0.25*(xh+xl) = x into the psum, so the drain is a pure copy
            # (split DVE/ACT so the two drain streams run in parallel)
            nc.tensor.matmul(
                op_, lhsT=identq, rhs=xhl[:, :, ot, nsl], start=False,
                stop=True, perf_mode=DR, skip_group_check=True,
            )
            if ot >= 2:
                nc.scalar.copy(out=obt[:, ot, :], in_=op_)
            else:
                nc.vector.tensor_copy(obt[:, ot, :], op_)
            if store_split and ot == 1:
                nc.sync.dma_start(out=out3[:, 0:2, nsl], in_=obt[:, 0:2, :])
        if 3 in ots:
            if store_split:
                nc.scalar.dma_start(out=out3[:, 2:4, nsl], in_=obt[:, 2:4, :])
            else:
                (queue or nc.sync).dma_start(out=out3[:, :, nsl], in_=obt)
        return obt

    # phase-1 score pull-in: fill PE gaps left by the x-DMA cadence with
    # slice-0/1 score matmuls (their exps + pair sums trail on ACT/DVE)
    pull = [(0, 0), (0, 1), (1, 0)]
    for mt in range(1, 7):
        pull += [(2, mt - 1), (1, mt), (0, mt + 1)]
    pull += [(1, 6), (2, 6)]
    pulled = 0

    def pump_scores(ns, budget):
        nonlocal pulled
        done = 0
        while pulled < len(pull) and done < budget:
            i, mt = pull[pulled]
            if i > ns - 1 or mt > ns - 1:
                break
            emit_score(i, mt)
            if mt % 2 == 1:
                emit_fs1(i, mt // 2)
            pulled += 1
            done += 1

    # ---- phase 1: projections (hi-lo fp8 DoubleRow) + pooling + gT --------
    terms = ((wpth, 0), (wpth, 1), (wptl, 0))

    def emit_gt(tp, ns):
        # transpose slice ns's pooled g columns (m-tile ns) into gT[ns];
        # emitted one slice late so the PE never waits on the DVE pooling.
        # Drains on DVE: ACT is saturated with theta drains + entry exps.
        msl = slice(ns * P, (ns + 1) * P)
        for i in range(2):
            t = tp.tile([P, P], BF16, name="tp", tag=f"tp{i}")
            nc.tensor.transpose(t, g01[:, i, msl], ident_b)
            nc.vector.tensor_copy(gT[ns][:, i * P : (i + 1) * P], t)

    with tc.tile_pool(name="ppsum", bufs=1, space="PSUM") as pp, tc.tile_pool(
        name="tpsum", bufs=1, space="PSUM"
    ) as tp:
        for ns in range(NS):
            msl = slice(ns * P, (ns + 1) * P)
            nsl = slice(ns * SL, (ns + 1) * SL)
            # one 2-bank psum tile for both g out-tiles: a single fused
            # pooling reduce drains it, keeping the DVE under the proj pace
            ps_g = pp.tile([P, 2, SL], F32, name="ppg", tag="ppg", bufs=1)
            ps_tp = pp.tile([P, SL], F32, name="pp0", tag="pp0", bufs=1)
            mms = [ps_g[:, 0, :], ps_g[:, 1, :], ps_tp]
            # g-first matmul order: their psums are ready first and gate
            # this slice's gT transposes
            mt_order = (0, 1, 2)
            if ns <= 1:
                # early slices: arrival-major so each weight/x chunk arrival
                # during the startup stream unlocks matmuls incrementally
                order = [
                    (ti, q, mt)
                    for ti in range(3)
                    for q in range(2)
                    for mt in mt_order
                ]
            else:
                order = [
                    (ti, q, mt)
                    for mt in mt_order
                    for ti in range(3)
                    for q in range(2)
                ]
            # psum out-tile mt: 0,1 = g halves (wproj cols 128..384), 2 = theta+phi
            wcol = (1, 2, 0)
            for ti, q, mt in order:
                wt, hl = terms[ti]
                wc = wcol[mt]
                nc.tensor.matmul(
                    mms[mt],
                    lhsT=wt[:, 2 * q : 2 * q + 2, wc * P : (wc + 1) * P],
                    rhs=xhl[:, hl, 2 * q : 2 * q + 2, nsl],
                    start=(ti == 0 and q == 0),
                    stop=(ti == 2 and q == 1),
                    perf_mode=DR,
                    skip_group_check=True,
                )
            nc.vector.tensor_reduce(
                out=g01[:, :, msl],
                in_=ps_g.rearrange(
                    "p i (h2 dy w2 dx) -> p i h2 w2 dy dx",
                    h2=4, dy=2, w2=32, dx=2,
                ),
                axis=AX.XY,
                op=ALU.max,
            )
            nc.vector.tensor_reduce(
                out=phi[:, msl],
                in_=_pool_view(ps_tp[C8:P, :]),
                axis=AX.XY,
                op=ALU.max,
            )
            # theta drain descales the 64x hi-lo projection scale (4*16)^2
            if ns == NS - 1:
                # last slice on DVE to keep ACT free for the entry exps
                nc.vector.tensor_scalar(
                    out=theta[:, nsl], in0=ps_tp[0:C8, :], scalar1=2.0**-12,
                    scalar2=None, op0=ALU.mult,
                )
            else:
                nc.scalar.activation(
                    theta[:, nsl], ps_tp[0:C8, :], ACTF.Copy, scale=2.0**-12
                )
            if ns >= 1:
                emit_gt(tp, ns - 1)
            pump_scores(ns, 3)
        emit_gt(tp, NS - 1)

    # ---- phase 2: softmax / attend / project ---------------------------
    with tc.tile_pool(name="qpsum", bufs=1, space="PSUM") as qp:
        # slice-0 leftovers first: ap(0) mt7 is the earliest consumer of the
        # entry exp backlog on ACT
        emit_scores_full(0)
        emit_fs_rest(0, eng=nc.vector)
        for i in range(NS):
            last = i == NS - 1
            # out(i-1) ot3 reuses ot0's psum bank, which frees only after the
            # DVE residual add drains (~1.6us); interleave half of ap(i) so
            # the PE never waits on that ring. out(NS-2) was already flushed
            # at the end of the previous iteration.
            if 1 <= i < NS - 1:
                obt = emit_out(qp, i - 1, 0, SL, ots=(0,))
            if not last:
                ap = emit_ap(qp, i, 0, SL)
                # DVE order matters: recip+att-muls for slice i right after
                # ob0, ahead of the remaining residual adds and the fs tree,
                # so out(i) next iteration is not late on the att tiles
                emit_norm(i, ap, 0, SL)
                if i >= 1:
                    emit_out(qp, i - 1, 0, SL, ots=(1, 2, 3), obt=obt)
                if i == 0:
                    emit_scores_full(1)
                if i + 2 < NS:
                    emit_scores_full(i + 2)
                emit_fs_rest(i + 1)
                if i == NS - 2:
                    # no scores left to overlap: flush out(6) now so its
                    # store clears the DMA engines before the drain chunks
                    obt = emit_out(
                        qp, i, 0, SL, ots=(0, 1, 2), store_split=True,
                    )
                    emit_out(qp, i, 0, SL, ots=(3,), obt=obt,
                             store_split=True)
            else:
                # drain: narrowing trailing chunks so the final store's data
                # is ready as early as possible (store latency ~3.2us is the
                # structural tail); chunk 1 borrows the idle score psum banks
                chunks = ((0, 256), (256, 128), (384, 128))
                aps = [None] * 3
                aps[0] = emit_ap(qp, i, *chunks[0])
                aps[1] = emit_ap(qp, i, *chunks[1], tags=("s0", "s1"), pool=spool)
                emit_norm(i, aps[0], *chunks[0])
                att0 = ATT[i]
                # norm(c1) ahead of c0's drain copies on the in-order DVE
                # queue, so the final out matmuls are not stuck behind them
                emit_norm(i, aps[1], *chunks[1])
                att1 = ATT[i]
                ATT[i] = att0
                emit_out(qp, i, *chunks[0], ots=(0, 1, 2, 3))
                aps[2] = emit_ap(qp, i, *chunks[2])
                ATT[i] = att1
                emit_out(qp, i, *chunks[1], ots=(0, 1, 2, 3), queue=nc.scalar)
                emit_norm(i, aps[2], *chunks[2])
                emit_out(qp, i, *chunks[2], ots=(0, 1, 2, 3),
                         store_split=True)


def build_nc():
    nc = bass.Bass(target_bir_lowering=False, trn_type="TRN2")
    with tile.TileContext(nc) as tc:
        with ExitStack() as ctx:
            emit(nc, tc, ctx)
    bass_rust.generate_event_semaphores(nc)
    return nc


def kernel(x, w_theta, w_phi, w_g, w_o, gamma):
    import ml_dtypes

    x = np.asarray(x, dtype=np.float32)
    B = x.shape[0]
    SX, SW = 4.0, 16.0
    wproj = np.concatenate(
        [np.asarray(w_theta).T, np.asarray(w_phi).T, np.asarray(w_g).T], axis=1
    ).astype(np.float32)
    wps = SW * wproj
    wph = wps.astype(ml_dtypes.float8_e4m3)
    wpl = (wps - wph.astype(np.float32)).astype(ml_dtypes.float8_e4m3)
    wph = np.ascontiguousarray(wph)
    wpl = np.ascontiguousarray(wpl)
    wo_t = np.ascontiguousarray(
        (np.float32(gamma) * np.asarray(w_o)).T.astype(ml_dtypes.bfloat16)
    )

    nc = build_nc()
    in_maps = []
    for b in range(B):
        xb = x[b].reshape(C, N)
        xs = SX * xb
        xh = xs.astype(ml_dtypes.float8_e4m3)
        xl = (xs - xh.astype(np.float32)).astype(ml_dtypes.float8_e4m3)
        in_maps.append(
            {
                "xh": np.ascontiguousarray(xh),
                "xl": np.ascontiguousarray(xl),
                "xr": np.ascontiguousarray(xb.astype(ml_dtypes.bfloat16)),
                "wph": wph,
                "wpl": wpl,
                "wo": wo_t,
            }
        )
    # retry: rare transient NRT_EXEC_UNIT_UNRECOVERABLE from stale device
    # state clears on re-execution
    last_err = None
    for attempt in range(3):
        try:
            res = run_bass_kernel_spmd(nc, in_maps, core_ids=list(range(B)))
            break
        except Exception as e:  # noqa: BLE001
            last_err = e
            time.sleep(2.0)
    else:
        raise last_err
    out = np.stack(
        [res.results[b]["out"].reshape(C, 64, 64) for b in range(B)]
    ).astype(np.float32)
    return out


# revision 23
# speedup vs baseline: 1.0356x; 1.0356x over previous
"""SAGAN-style self-attention block on 8 trn2 NeuronCores.

Full inputs: x [8, 512, 64, 64], w_theta [64, 512], w_phi [64, 512],
w_g [256, 512], w_o [512, 256], gamma scalar.

Sharding: data-parallel over batch — one batch item per core. Each core runs
an identical Bass program over its own x[b]; weights are replicated.

Per-core math (C=512, n=H*W=4096, m=n/4=1024):
  theta = w_theta @ x            [64, 4096]
  phi   = pool2(w_phi @ x)       [64, 1024]
  g     = pool2(w_g @ x)         [256, 1024]
  S^T   = phi^T @ theta          [1024, 4096]  (scores, transposed layout)
  E     = exp(S^T)               (bf16; no max-subtraction: |S| < ~51)
  Z     = ones^T @ sum-tree(E)   (pair tree: 4 bf16 adds on gpsimd, 3 on
                                  DVE; broadcast row sums via one PE matmul)
  att   = (g @ E) / Z            [256, 4096]   (bf16 after normalize)
  out   = (gamma*w_o) @ att + x  [512, 4096]

The projection runs as fp8 (e4m3) DoubleRow matmuls at 0.5 cycles/column:
x and w_proj are split host-side into hi+lo e4m3 pairs (w scaled by 16, x by
4 so the residuals stay in e4m3's normal range); the three cross terms
wh@xh + wh@xl + wl@xh recover bf16-level accuracy at 75% of the bf16 matmul
cost. The 64x scale cancels exactly: theta drains through an ACT copy with
scale 2^-12, and the Z ones-matmul uses 64.0 so att = (64 g E)/(64 Z).
Scores/attend/out matmuls stay bf16 (fp8 there fails the rel-err budget:
softmax logit noise flips argmaxes at sigma_S ~ 8). g transposes ride the
DMA xbar (dma_start_transpose) instead of PE+DVE.

The residual add uses a separate bf16 copy of x; in the drain (slices 6-7)
half the residuals ride the PE via an identity matmul so the psum drain
becomes an ACT copy. Stores are batched [128,4,512] 3D DMAs.
"""

import time
from contextlib import ExitStack

import numpy as np

import bass_rust
import concourse.bass as bass
import concourse.mybir as mybir
import concourse.tile as tile
from concourse.bass_utils import run_bass_kernel_spmd
from concourse.masks import make_identity

P = 128
C = 512  # channels
C8 = 64  # theta/phi channels
C2 = 256  # g channels
N = 4096  # H*W
M = 1024  # pooled spatial
NS = 8  # n-slices
SL = 512  # n-slice width
MT = 8  # m-tiles of 128
F32 = mybir.dt.float32
BF16 = mybir.dt.bfloat16
E4 = mybir.dt.float8e4
AX = mybir.AxisListType
ALU = mybir.AluOpType
ACTF = mybir.ActivationFunctionType
DR = mybir.MatmulPerfMode.DoubleRow


def _pool_view(ap, w=SL):
    """[p, w] slice of the conv output -> 5D maxpool view [p, h2, w2, dy, dx].

    Within an n-slice of 512 = 8 image rows: local n = (2*h2+dy)*64 + 2*w2+dx.
    """
    return ap.rearrange(
        "p (h2 dy w2 dx) -> p h2 w2 dy dx", h2=w // 128, dy=2, w2=32, dx=2
    )


def emit(nc, tc, ctx):
    xh_d = nc.dram_tensor("xh", [C, N], E4, kind="ExternalInput")
    xl_d = nc.dram_tensor("xl", [C, N], E4, kind="ExternalInput")
    wph_d = nc.dram_tensor("wph", [C, 384], E4, kind="ExternalInput")
    wpl_d = nc.dram_tensor("wpl", [C, 384], E4, kind="ExternalInput")
    wo_d = nc.dram_tensor("wo", [C2, C], BF16, kind="ExternalInput")
    out_d = nc.dram_tensor("out", [C, N], BF16, kind="ExternalOutput")

    persist = ctx.enter_context(tc.tile_pool(name="persist", bufs=1))

    # weights on the scalar (ACT) DMA queue in chunks so chunk 0 lands early:
    # the first projection matmul only waits on it + the first x chunk
    wpth = persist.tile([P, 4, 384], E4, name="wpth")
    wptl = persist.tile([P, 4, 384], E4, name="wptl")
    wph3 = wph_d.ap().rearrange("(k p) o -> p k o", k=4)
    nc.scalar.dma_start(out=wpth[:, 0:2, :], in_=wph3[:, 0:2, :])
    wot = persist.tile([P, 2, C], BF16, name="wot")

    ones_b = persist.tile([P, P], BF16)
    nc.vector.memset(ones_b, 64.0)
    ident_f = persist.tile([P, P], F32)
    make_identity(nc, ident_f)
    ident_b = persist.tile([P, P], BF16)
    nc.vector.tensor_copy(ident_b, ident_f)
    # fp8 DoubleRow identity pair [0.25 I; 0.25 I]: one DR matmul against the
    # [xh; xl] k-pair reconstructs the residual x into the out psum exactly
    # (0.25 undoes the 4x host-side scale on x)
    identq = persist.tile([P, 2, P], E4, name="identq")
    for i in range(2):
        nc.vector.tensor_scalar(
            out=identq[:, i, :], in0=ident_f, scalar1=0.25, scalar2=None,
            op0=ALU.mult,
        )

    # score psum pool lives across both phases (slice 0/1 scores start in
    # phase 1)
    spool = ctx.enter_context(tc.tile_pool(name="spsum", bufs=3, space="PSUM"))
    etp = ctx.enter_context(tc.tile_pool(name="et", bufs=4))
    fsp = ctx.enter_context(tc.tile_pool(name="fs", bufs=2))
    miscp = ctx.enter_context(tc.tile_pool(name="misc", bufs=2))

    # Warm-up for the otherwise-idle startup window (PE waits ~4us for the
    # first weight+x data): dummy exp preloads the ACT exp table, and a burst
    # of matmuls on constant data ramps the PE clock (HAM).
    actwarm = persist.tile([P, 1], F32)
    nc.scalar.activation(actwarm, ident_f[:, 0:1], ACTF.Exp)
    for wi in range(5):
        wt_ = spool.tile([P, P], F32, name="warm", tag="s0", bufs=1)
        nc.tensor.matmul(wt_, lhsT=ones_b, rhs=ones_b, start=True, stop=True)

    # x loads: hi and lo fp8 streams in one [P, 2(hi/lo), 4(k), N] tile so a
    # DoubleRow ap can pair them. hi rides the sync (SP) queue, lo the scalar
    # queue behind the first weight chunk; HWDGE descriptor-gen is a shared
    # serial ~630ns/DMA resource, so loads are batched 2 slices per DMA once
    # the pipeline is primed.
    xhl = persist.tile([P, 2, 4, N], E4, name="xhl")
    xh3 = xh_d.ap().rearrange("(k p) n -> p k n", k=4)
    xl3 = xl_d.ap().rearrange("(k p) n -> p k n", k=4)
    for cc in range(2):
        nc.sync.dma_start(
            out=xhl[:, 0, 2 * cc : 2 * cc + 2, 0:SL],
            in_=xh3[:, 2 * cc : 2 * cc + 2, 0:SL],
        )
    nc.sync.dma_start(out=xhl[:, 0, :, SL : 2 * SL], in_=xh3[:, :, SL : 2 * SL])
    for q in range(1, 4):
        qs = slice(2 * q * SL, 2 * (q + 1) * SL)
        nc.sync.dma_start(out=xhl[:, 0, :, qs], in_=xh3[:, :, qs])
        nc.sync.dma_start(out=xhl[:, 1, :, qs], in_=xl3[:, :, qs])
    nc.sync.dma_start(
        out=wot, in_=wo_d.ap().rearrange("(ct p) c -> p ct c", ct=2)
    )
    # scalar queue interleaves the remaining weight chunks with the early
    # xl slices so every projection term unblocks as soon as possible
    nc.scalar.dma_start(out=xhl[:, 1, :, 0:SL], in_=xl3[:, :, 0:SL])
    nc.scalar.dma_start(out=wpth[:, 2:4, :], in_=wph3[:, 2:4, :])
    nc.scalar.dma_start(
        out=wptl, in_=wpl_d.ap().rearrange("(k p) o -> p k o", k=4)
    )
    nc.scalar.dma_start(out=xhl[:, 1, :, SL : 2 * SL], in_=xl3[:, :, SL : 2 * SL])

    out3 = out_d.ap().rearrange("(ot p) n -> p ot n", ot=4)

    theta = persist.tile([C8, N], BF16)
    phi = persist.tile([C8, M], BF16)
    g01 = persist.tile([P, 2, M], BF16, name="g01")
    gT = [persist.tile([P, C2], BF16, name=f"gT{mt}") for mt in range(MT)]

    ET = [[None] * MT for _ in range(NS)]
    FS1 = [[None] * 4 for _ in range(NS)]  # pair sums
    ZB = [None] * NS  # broadcast row sums
    ATT = [[None, None] for _ in range(NS)]
    score_done = [[False] * MT for _ in range(NS)]
    fs1_done = [[False] * 4 for _ in range(NS)]

    def emit_score(i, mt):
        # S^T tile [m 128, n 512] = phi_mt^T @ theta_i  (K = 64 channels)
        sp = spool.tile([P, SL], F32, name="sp", tag=f"s{mt % 3}", bufs=1)
        nc.tensor.matmul(
            sp,
            lhsT=phi[:, mt * P : (mt + 1) * P],
            rhs=theta[:, i * SL : (i + 1) * SL],
            start=True,
            stop=True,
            skip_group_check=True,
        )
        et = etp.tile([P, SL], BF16, name="et", tag=f"et{mt}")
        nc.scalar.activation(et, sp, ACTF.Exp)
        ET[i][mt] = et
        score_done[i][mt] = True

    def emit_fs1(i, j, eng=None):
        # pair sums on the Pool engine (gpsimd): it cannot touch PSUM, so
        # this SBUF-only stage is the one piece of DVE work it can take
        t = fsp.tile([P, SL], BF16, name="fs1", tag=f"f{j}")
        (eng or nc.gpsimd).tensor_add(t, ET[i][2 * j], ET[i][2 * j + 1])
        FS1[i][j] = t
        fs1_done[i][j] = True

    def emit_fs_rest(i, eng=None):
        # finish the pair-sum tree and hand the total to the Pool engine for
        # the cross-partition broadcast sum
        for j in range(4):
            if not fs1_done[i][j]:
                emit_fs1(i, j, eng)
        h0 = fsp.tile([P, SL], BF16, name="fs2", tag="h0")
        h1 = fsp.tile([P, SL], BF16, name="fs2", tag="h1")
        nc.vector.tensor_add(h0, FS1[i][0], FS1[i][1])
        nc.vector.tensor_add(h1, FS1[i][2], FS1[i][3])
        hh = fsp.tile([P, SL], BF16, name="fs3", tag="hh")
        nc.vector.tensor_add(hh, h0, h1)
        # broadcast row sums via ones-matmul (cross-partition reduce); the
        # 64.0 ones value cancels the 64x projection scale on g. rides a
        # score psum bank that is idle at this point of the iteration
        zb = spool.tile([P, SL], F32, name="zb", tag="s2", bufs=1)
        nc.tensor.matmul(
            zb, lhsT=ones_b, rhs=hh, start=True, stop=True,
            skip_group_check=True,
        )
        ZB[i] = zb

    def emit_scores_full(i):
        for mt in range(MT):
            if not score_done[i][mt]:
                emit_score(i, mt)

    def emit_ap(qp, i, lo, w, mts=range(MT), ap=None, tags=("a0", "a1"), pool=None):
        esl = slice(lo, lo + w)
        if ap is None:
            ap = [
                (pool or qp).tile([P, w], F32, name="ap", tag=tags[ct], bufs=1)
                for ct in range(2)
            ]
        for mt in mts:
            st, sp_ = (mt == 0), (mt == MT - 1)
            for ct in range(2):
                nc.tensor.matmul(
                    ap[ct],
                    lhsT=gT[mt][:, ct * P : (ct + 1) * P],
                    rhs=ET[i][mt][:, esl],
                    start=st,
                    stop=sp_,
                    skip_group_check=True,
                )
        return ap

    def emit_norm(i, ap, lo, w):
        esl = slice(lo, lo + w)
        rinv = miscp.tile([P, w], F32, name="rinv", tag="rinv")
        nc.vector.reciprocal(rinv, ZB[i][:, esl])
        att = [None, None]
        # ct0 first: the out matmuls consume ct0 first
        for ct in (0, 1):
            t = miscp.tile([P, w], BF16, name="att", tag=f"att{ct}")
            nc.vector.tensor_mul(t, ap[ct], rinv)
            att[ct] = t
        ATT[i] = att

    def emit_out(qp, i, lo, w, ots, obt=None, queue=None, store_split=False):
        # project + residual for columns [i*SL+lo, ..+w); the batched store
        # goes out once ot 3 is in (or per ot-pair when store_split)
        nsl = slice(i * SL + lo, i * SL + lo + w)
        att = ATT[i]
        if obt is None:
            # 4-deep: a slice's batched store completes ~1.5 iterations after
            # its compute, and the drain chunks must not wait on it
            tag = "ob" if w == SL else "obc"
            obt = miscp.tile([P, 4, w], BF16, name="ob", tag=tag, bufs=3)
        for ot in ots:
            op_ = qp.tile([P, w], F32, name="op", tag="o", bufs=3)
            for ct in range(2):
                nc.tensor.matmul(
                    op_,
                    lhsT=wot[:, ct, ot * P : (ot + 1) * P],
                    rhs=att[ct],
                    start=(ct == 0),
                    stop=False,
                    skip_group_check=True,
                )
            # residual: one fp8 DoubleRow identity matmul accumulates
            # 0.25*(xh+xl) = x into the psum, so the drain is a pure copy
            # (split DVE/ACT so the two drain streams run in parallel)
            nc.tensor.matmul(
                op_, lhsT=identq, rhs=xhl[:, :, ot, nsl], start=False,
                stop=True, perf_mode=DR, skip_group_check=True,
            )
            if ot >= 2:
                nc.scalar.copy(out=obt[:, ot, :], in_=op_)
            else:
                nc.vector.tensor_copy(obt[:, ot, :], op_)
            if store_split and ot == 1:
                nc.sync.dma_start(out=out3[:, 0:2, nsl], in_=obt[:, 0:2, :])
        if 3 in ots:
            if store_split:
                nc.scalar.dma_start(out=out3[:, 2:4, nsl], in_=obt[:, 2:4, :])
            else:
                (queue or nc.sync).dma_start(out=out3[:, :, nsl], in_=obt)
        return obt

    # phase-1 score pull-in: fill PE gaps left by the x-DMA cadence with
    # slice-0/1 score matmuls (their exps + pair sums trail on ACT/DVE)
    pull = [(0, 0), (0, 1), (1, 0)]
    for mt in range(1, 7):
        pull += [(2, mt - 1), (1, mt), (0, mt + 1)]
    pull += [(1, 6), (2, 6)]
    pulled = 0

    def pump_scores(ns, budget):
        nonlocal pulled
        done = 0
        while pulled < len(pull) and done < budget:
            i, mt = pull[pulled]
            if i > ns - 1 or mt > ns - 1:
                break
            emit_score(i, mt)
            if mt % 2 == 1:
                emit_fs1(i, mt // 2)
            pulled += 1
            done += 1

    # ---- phase 1: projections (hi-lo fp8 DoubleRow) + pooling + gT --------
    terms = ((wpth, 0), (wpth, 1), (wptl, 0))

    def emit_gt(tp, ns):
        # transpose slice ns's pooled g columns (m-tile ns) into gT[ns];
        # emitted one slice late so the PE never waits on the DVE pooling.
        # Drains on DVE: ACT is saturated with theta drains + entry exps.
        msl = slice(ns * P, (ns + 1) * P)
        for i in range(2):
            t = tp.tile([P, P], BF16, name="tp", tag=f"tp{i}")
            nc.tensor.transpose(t, g01[:, i, msl], ident_b)
            nc.vector.tensor_copy(gT[ns][:, i * P : (i + 1) * P], t)

    with tc.tile_pool(name="ppsum", bufs=1, space="PSUM") as pp, tc.tile_pool(
        name="tpsum", bufs=1, space="PSUM"
    ) as tp:
        for ns in range(NS):
            msl = slice(ns * P, (ns + 1) * P)
            nsl = slice(ns * SL, (ns + 1) * SL)
            # one 2-bank psum tile for both g out-tiles: a single fused
            # pooling reduce drains it, keeping the DVE under the proj pace
            ps_g = pp.tile([P, 2, SL], F32, name="ppg", tag="ppg", bufs=1)
            ps_tp = pp.tile([P, SL], F32, name="pp0", tag="pp0", bufs=1)
            mms = [ps_g[:, 0, :], ps_g[:, 1, :], ps_tp]
            # g-first matmul order: their psums are ready first and gate
            # this slice's gT transposes
            mt_order = (0, 1, 2)
            if ns <= 1:
                # early slices: arrival order — (wh q0, xl q0, wh q1, xl q1,
                # wl q0/q1) matches the DMA landing sequence during startup
                order = [
                    (ti, q, mt)
                    for ti, q in ((0, 0), (1, 0), (0, 1), (1, 1), (2, 0), (2, 1))
                    for mt in mt_order
                ]
            else:
                order = [
                    (ti, q, mt)
                    for mt in mt_order
                    for ti in range(3)
                    for q in range(2)
                ]
            # psum out-tile mt: 0,1 = g halves (wproj cols 128..384), 2 = theta+phi
            wcol = (1, 2, 0)
            for ti, q, mt in order:
                wt, hl = terms[ti]
                wc = wcol[mt]
                nc.tensor.matmul(
                    mms[mt],
                    lhsT=wt[:, 2 * q : 2 * q + 2, wc * P : (wc + 1) * P],
                    rhs=xhl[:, hl, 2 * q : 2 * q + 2, nsl],
                    start=(ti == 0 and q == 0),
                    stop=(ti == 2 and q == 1),
                    perf_mode=DR,
                    skip_group_check=True,
                )
            nc.vector.tensor_reduce(
                out=g01[:, :, msl],
                in_=ps_g.rearrange(
                    "p i (h2 dy w2 dx) -> p i h2 w2 dy dx",
                    h2=4, dy=2, w2=32, dx=2,
                ),
                axis=AX.XY,
                op=ALU.max,
            )
            nc.vector.tensor_reduce(
                out=phi[:, msl],
                in_=_pool_view(ps_tp[C8:P, :]),
                axis=AX.XY,
                op=ALU.max,
            )
            # theta drain descales the 64x hi-lo projection scale (4*16)^2
            if ns == NS - 1:
                # last slice on DVE to keep ACT free for the entry exps
                nc.vector.tensor_scalar(
                    out=theta[:, nsl], in0=ps_tp[0:C8, :], scalar1=2.0**-12,
                    scalar2=None, op0=ALU.mult,
                )
            else:
                nc.scalar.activation(
                    theta[:, nsl], ps_tp[0:C8, :], ACTF.Copy, scale=2.0**-12
                )
            if ns >= 1:
                emit_gt(tp, ns - 1)
            pump_scores(ns, 3)
        emit_gt(tp, NS - 1)

    # ---- phase 2: softmax / attend / project ---------------------------
    with tc.tile_pool(name="qpsum", bufs=1, space="PSUM") as qp:
        # slice-0 leftovers first: ap(0) mt7 is the earliest consumer of the
        # entry exp backlog on ACT
        emit_scores_full(0)
        emit_fs_rest(0, eng=nc.vector)
        for i in range(NS):
            last = i == NS - 1
            # out(i-1) ot3 reuses ot0's psum bank, which frees only after the
            # DVE residual add drains (~1.6us); interleave half of ap(i) so
            # the PE never waits on that ring. out(NS-2) was already flushed
            # at the end of the previous iteration.
            if 1 <= i < NS - 1:
                obt = emit_out(qp, i - 1, 0, SL, ots=(0,))
            if not last:
                ap = emit_ap(qp, i, 0, SL)
                # DVE order matters: recip+att-muls for slice i right after
                # ob0, ahead of the remaining residual adds and the fs tree,
                # so out(i) next iteration is not late on the att tiles
                emit_norm(i, ap, 0, SL)
                if i >= 1:
                    emit_out(qp, i - 1, 0, SL, ots=(1, 2, 3), obt=obt)
                if i == 0:
                    emit_scores_full(1)
                if i + 2 < NS:
                    emit_scores_full(i + 2)
                emit_fs_rest(i + 1)
                if i == NS - 2:
                    # no scores left to overlap: flush out(6) now so its
                    # store clears the DMA engines before the drain chunks
                    obt = emit_out(
                        qp, i, 0, SL, ots=(0, 1, 2), store_split=True,
                    )
                    emit_out(qp, i, 0, SL, ots=(3,), obt=obt,
                             store_split=True)
            else:
                # drain: narrowing trailing chunks so the final store's data
                # is ready as early as possible (store latency ~3.2us is the
                # structural tail); chunk 1 borrows the idle score psum banks
                chunks = ((0, 256), (256, 128), (384, 128))
                aps = [None] * 3
                aps[0] = emit_ap(qp, i, *chunks[0])
                aps[1] = emit_ap(qp, i, *chunks[1], tags=("s0", "s1"), pool=spool)
                emit_norm(i, aps[0], *chunks[0])
                att0 = ATT[i]
                # norm(c1) ahead of c0's drain copies on the in-order DVE
                # queue, so the final out matmuls are not stuck behind them
                emit_norm(i, aps[1], *chunks[1])
                att1 = ATT[i]
                ATT[i] = att0
                emit_out(qp, i, *chunks[0], ots=(0, 1, 2, 3))
                aps[2] = emit_ap(qp, i, *chunks[2])
                ATT[i] = att1
                emit_out(qp, i, *chunks[1], ots=(0, 1, 2, 3), queue=nc.scalar)
                emit_norm(i, aps[2], *chunks[2])
                emit_out(qp, i, *chunks[2], ots=(0, 1, 2, 3),
                         store_split=True)


def build_nc():
    nc = bass.Bass(target_bir_lowering=False, trn_type="TRN2")
    with tile.TileContext(nc) as tc:
        with ExitStack() as ctx:
            emit(nc, tc, ctx)
    bass_rust.generate_event_semaphores(nc)
    return nc


def kernel(x, w_theta, w_phi, w_g, w_o, gamma):
    import ml_dtypes

    x = np.asarray(x, dtype=np.float32)
    B = x.shape[0]
    SX, SW = 4.0, 16.0
    wproj = np.concatenate(
        [np.asarray(w_theta).T, np.asarray(w_phi).T, np.asarray(w_g).T], axis=1
    ).astype(np.float32)
    wps = SW * wproj
    wph = wps.astype(ml_dtypes.float8_e4m3)
    wpl = (wps - wph.astype(np.float32)).astype(ml_dtypes.float8_e4m3)
    wph = np.ascontiguousarray(wph)
    wpl = np.ascontiguousarray(wpl)
    wo_t = np.ascontiguousarray(
        (np.float32(gamma) * np.asarray(w_o)).T.astype(ml_dtypes.bfloat16)
    )

    nc = build_nc()
    in_maps = []
    for b in range(B):
        xb = x[b].reshape(C, N)
        xs = SX * xb
        xh = xs.astype(ml_dtypes.float8_e4m3)
        xl = (xs - xh.astype(np.float32)).astype(ml_dtypes.float8_e4m3)
        in_maps.append(
            {
                "xh": np.ascontiguousarray(xh),
                "xl": np.ascontiguousarray(xl),
                "xr": np.ascontiguousarray(xb.astype(ml_dtypes.bfloat16)),
                "wph": wph,
                "wpl": wpl,
                "wo": wo_t,
            }
        )
    # retry: rare transient NRT_EXEC_UNIT_UNRECOVERABLE from stale device
    # state clears on re-execution
    last_err = None
    for attempt in range(3):
        try:
            res = run_bass_kernel_spmd(nc, in_maps, core_ids=list(range(B)))
            break
        except Exception as e:  # noqa: BLE001
            last_err = e
            time.sleep(2.0)
    else:
        raise last_err
    out = np.stack(
        [res.results[b]["out"].reshape(C, 64, 64) for b in range(B)]
    ).astype(np.float32)
    return out
